# revision 40
# baseline (speedup 1.0000x reference)
"""Trainium2 Bass kernel for Llama-style GQA attention (B=2,S=2048,H=4096,NH=32,NKV=8,HD=128).

Sharding: tensor-parallel over heads for QKV+attention (core c owns Q-heads
4c..4c+3 and GQA KV-head c). For the output projection, cores exchange
attention outputs with a single bf16 AllToAll (each core ends with all 4096
features for its 512 tokens) and then apply the FULL Wo locally, streamed
from HBM. This moves ~4 MB per core over the interconnect instead of the
~67 MB fp32 partial-sum ReduceScatter.

Three GEMM-dense phases keep the PE array continuously busy (p-state ramp):
  1) QKV projection + rope for all 8 token slices
  2) causal attention, software-pipelined across (slice, head) tiles
  3) AllToAll + output projection with Wo streamed from HBM
"""

import math
import os
from contextlib import ExitStack

import numpy as np

B, S, H = 2, 2048, 4096
NH, NKV, HD = 32, 8, 128
THETA = 1000000.0
NCORES = 8
QH = NH // NCORES            # 4 q-heads per core
TOK = B * S                  # 4096 tokens (flattened batch*seq)
QO = QH * HD                 # 512 q-out dims per core
TT = TOK // 128              # 32 token tiles of 128
TS = TOK // 512              # 8 token slices of 512
SB = S // 512                # 4 q-slices of 512 per batch
KTB = S // 128               # 16 k-tiles of 128 per batch
NFC = NH                     # 32 feature chunks of 128 (full Wo contraction)

LAST_EXEC_NS = None
LAST_RESULT = None

_compiled = {}


def _build():
    import concourse.bass as bass
    import concourse.mybir as mybir
    import concourse.tile as tile
    from concourse import bacc

    f32 = mybir.dt.float32
    bf = mybir.dt.bfloat16
    nc = bacc.Bacc("TRN2", target_bir_lowering=False, debug=False,
                   num_devices=NCORES)

    def inp(name, shape, dt=f32):
        return nc.dram_tensor(name, shape, dt, kind="ExternalInput").ap()

    # hidden transposed and host-pre-tiled: xTt[ti] is a contiguous
    # (H, 512) block for token slice ti -> single-burst DMA tiles
    xTt = inp("xTt", (TS, H, 512), bf)
    # QKV weight shards host-permuted to SBUF-resident layout [p, tile, out]
    wqP = inp("wqP", (128, H // 128, QO), bf)
    wkP = inp("wkP", (128, H // 128, HD), bf)
    wvP = inp("wvP", (128, H // 128, HD), bf)
    bqP = inp("bqP", (128, QH))         # bq shard as [d, head]
    bkP = inp("bkP", (128, 1))
    bvP = inp("bvP", (128, 1))
    # FULL Wo^T blocked for streaming: [Hc, fq, d, k, o] with feature
    # f = (fq*2+k)*128 + d and output column o within chunk Hc
    woQ = inp("woQ", (H // 512, 16, 128, 2, 512), bf)
    cosT = inp("cosT", (HD, TOK), bf)
    sinT = inp("sinT", (HD, TOK), bf)
    rotM = inp("rotM", (HD, HD), bf)    # lhsT for rotate_half_interleaved
    ident = inp("ident", (128, 128), bf)
    ones1 = inp("ones1", (128, 1), bf)
    maskI = inp("maskI", (128, 4, 512), bf)  # causal diag-tile masks

    # core c's final output: rows 0:256 = batch-0 tokens c*256..(c+1)*256,
    # rows 256:512 = batch-1 tokens 2048+c*256..2048+(c+1)*256
    out = nc.dram_tensor("out", (512, H), f32, kind="ExternalOutput").ap()
    # Two-stage AllToAll of attention outputs (stage A: batch-0 slices 0-3,
    # fired mid-kernel; stage B: batch-1 slices 4-7, fired at the end and
    # hidden behind stage A's output projection). Owner j holds 256 tokens.
    a2aA_in = nc.dram_tensor("a2aA_in", (8, QO, 256), bf, kind="Internal").ap()
    a2aA_out = nc.dram_tensor("a2aA_out", (8, QO, 256), bf, kind="Internal").ap()
    a2aB_in = nc.dram_tensor("a2aB_in", (8, QO, 256), bf, kind="Internal").ap()
    a2aB_out = nc.dram_tensor("a2aB_out", (8, QO, 256), bf, kind="Internal").ap()

    inv_sqrt_hd = 1.0 / math.sqrt(HD)

    def mm(out_, lhsT, rhs, **kw):
        nc.tensor.matmul(out_, lhsT, rhs, **kw)

    with tile.TileContext(nc) as tc, ExitStack() as stk:
        # ---------------- constants + persistent activations ----------------
        cpool = stk.enter_context(tc.tile_pool(name="consts", bufs=1))
        apool = stk.enter_context(tc.tile_pool(name="acts", bufs=1))

        # Preamble loads go through the Activation HW-DGE queue so the xt
        # token stream (sync queue) is never stuck behind them. Weights are
        # interleaved per contraction chunk hi so the slice-0 QKV matmuls
        # start as soon as (wq|wk|wv)[hi=0] + xt tile 0 land (~2us), keeping
        # aggregate HBM demand under the per-core cap during startup.
        wq_res = apool.tile([128, H // 128, QO], bf)
        wk_res = apool.tile([128, H // 128, HD], bf)
        wv_res = apool.tile([128, H // 128, HD], bf)
        for hi in range(H // 128):
            nc.scalar.dma_start(wq_res[:, hi:hi + 1, :], wqP[:, hi:hi + 1, :])
            nc.scalar.dma_start(wk_res[:, hi:hi + 1, :], wkP[:, hi:hi + 1, :])
            nc.scalar.dma_start(wv_res[:, hi:hi + 1, :], wvP[:, hi:hi + 1, :])

        bq_sb = cpool.tile([128, QH], f32)
        nc.scalar.dma_start(bq_sb[:], bqP[:])
        bk_sb = cpool.tile([128, 1], f32)
        nc.scalar.dma_start(bk_sb[:], bkP[:])
        bv_sb = cpool.tile([128, 1], f32)
        nc.scalar.dma_start(bv_sb[:], bvP[:])
        cos_sb = cpool.tile([128, TOK], bf)
        nc.scalar.dma_start(cos_sb[:], cosT[:])
        sin_sb = cpool.tile([128, TOK], bf)
        nc.scalar.dma_start(sin_sb[:], sinT[:])
        rot_sb = cpool.tile([128, 128], bf)
        nc.scalar.dma_start(rot_sb[:], rotM[:])
        id_sb = cpool.tile([128, 128], bf)
        nc.scalar.dma_start(id_sb[:], ident[:])
        ones_sb = cpool.tile([128, 1], bf)
        nc.scalar.dma_start(ones_sb[:], ones1[:])
        mask_sb = cpool.tile([128, 4, 512], bf)
        nc.scalar.dma_start(mask_sb[:], maskI[:])

        KT = apool.tile([128, TOK], bf)          # K^T (rope'd)
        Vsb = apool.tile([128, TT, 128], bf)     # V in [t mod 128, t tile, d]
        QTs = apool.tile([128, TS, QH, 512], bf)  # all Q^T slices (rope'd)
        ATfA = apool.tile([128, NFC, 256], bf)   # post-A2A features, stage A
        ATfB = apool.tile([128, NFC, 256], bf)   # post-A2A features, stage B

        xp = stk.enter_context(tc.tile_pool(name="xts", bufs=4))
        vp = stk.enter_context(tc.tile_pool(name="vts", bufs=2))
        rp = stk.enter_context(tc.tile_pool(name="ropet", bufs=3))
        ap_ = stk.enter_context(tc.tile_pool(name="asb", bufs=7))
        anp = stk.enter_context(tc.tile_pool(name="atn", bufs=3))
        smp = stk.enter_context(tc.tile_pool(name="smalls", bufs=2))
        wop = stk.enter_context(tc.tile_pool(name="wos", bufs=6))
        osp = stk.enter_context(tc.tile_pool(name="ost", bufs=2))
        pp = stk.enter_context(tc.tile_pool(name="ps", bufs=8, space="PSUM"))

        def ps_tile(shape=(128, 512), dt=f32):
            return pp.tile(list(shape), dt, name="ps", tag="ps")

        # Wo streaming schedule: two full passes (one per A2A stage),
        # linear chunk list with lookahead prefetch + explicit pre-warm
        wo_chunks = [(p, Hc, fq) for p in range(2)
                     for Hc in range(H // 512) for fq in range(16)]
        wo_tiles = {}

        def wo_prefetch(idx):
            if idx < len(wo_chunks) and wo_chunks[idx] not in wo_tiles:
                pf = wop.tile([128, 2, 512], bf, name="wo_sb")
                nc.sync.dma_start(pf[:], woQ[wo_chunks[idx][1],
                                             wo_chunks[idx][2]])
                wo_tiles[wo_chunks[idx]] = pf

        def wo_get(idx):
            key = wo_chunks[idx]
            wo_sb = wo_tiles.pop(key, None)
            if wo_sb is None:
                wo_sb = wop.tile([128, 2, 512], bf, name="wo_sb")
                nc.sync.dma_start(wo_sb[:], woQ[key[1], key[2]])
            for la in (idx + 1, idx + 2, idx + 3, idx + 4):
                wo_prefetch(la)
            return wo_sb

        # =================== phase 1: QKV + rope (all slices) ===============
        def rope_slice(ti):
            t0 = ti * 512
            for q in range(QH + 1):
                ap_slice = KT[:, t0:t0 + 512] if q == QH else QTs[:, ti, q, :]
                rps = ps_tile()
                mm(rps[:], rot_sb[:], ap_slice, start=True, stop=True)
                t1 = rp.tile([128, 512], bf, name="t1")
                nc.vector.tensor_mul(t1[:], ap_slice, cos_sb[:, t0:t0 + 512])
                t2 = rp.tile([128, 512], bf, name="t2")
                nc.vector.tensor_mul(t2[:], rps[:], sin_sb[:, t0:t0 + 512])
                nc.vector.tensor_add(ap_slice, t1[:], t2[:])

        def vt_slice(ti, VTs):
            for s4 in range(4):
                g = ti * 4 + s4
                vps = pp.tile([128, 128], bf, name="vps", tag="ps")
                nc.tensor.transpose(vps[:], VTs[:, s4 * 128:(s4 + 1) * 128],
                                    id_sb[:])
                nc.scalar.copy(Vsb[:, g, :], vps[:])

        prev = None  # (ti, VTs) pending rope+transpose
        for ti in range(TS):
            t0 = ti * 512
            psq = [ps_tile() for _ in range(QH)]
            psk = ps_tile()
            psv = ps_tile()
            for hi in range(H // 128):
                h0 = hi * 128
                xt = xp.tile([128, 512], bf, name="xt")
                nc.sync.dma_start(xt[:], xTt[ti, h0:h0 + 128, :])
                st = (hi == 0)
                en = (hi == H // 128 - 1)
                for q in range(QH):
                    mm(psq[q][:], wq_res[:, hi, q * 128:(q + 1) * 128],
                       xt[:], start=st, stop=en)
                mm(psk[:], wk_res[:, hi, :], xt[:], start=st, stop=en)
                mm(psv[:], wv_res[:, hi, :], xt[:], start=st, stop=en)

            # bias add (per-partition) while draining PSUM
            VTs = vp.tile([128, 512], bf, name="VTs")
            for q in range(QH):
                nc.scalar.add(QTs[:, ti, q, :], psq[q][:], bq_sb[:, q:q + 1])
            nc.scalar.add(KT[:, t0:t0 + 512], psk[:], bk_sb[:, 0:1])
            nc.scalar.add(VTs[:], psv[:], bv_sb[:, 0:1])

            # rope + V-transpose of the PREVIOUS slice fills the PE while
            # this slice's PSUM drains on the scalar engine
            if prev is not None:
                rope_slice(prev[0])
                vt_slice(prev[0], prev[1])
            prev = (ti, VTs)
        rope_slice(prev[0])
        vt_slice(prev[0], prev[1])

        # ============ phase 2: causal attention, software-pipelined =========
        # global tile list over (slice, head, k-tile); at/dn matmuls lag the
        # score/exp stream by W tiles so the PE never waits on the Act engine
        W = 6
        tiles = []
        for ti in range(TS):
            b, j = ti // SB, ti % SB
            nk = 4 * j + 4
            for h in range(QH):
                for ki in range(nk):
                    tiles.append((ti, h, ki, nk, b, j))

        unit_ps = {}   # (ti,h) -> (at_ps, dn_ps)
        a_tiles = {}   # index -> exp'd score tile in SBUF

        def emit_front(n):
            ti, h, ki, nk, b, j = tiles[n]
            kg = b * KTB + ki
            sc_ps = ps_tile()
            mm(sc_ps[:], KT[:, kg * 128:(kg + 1) * 128],
               QTs[:, ti, h, :], start=True, stop=True)
            a_sb = ap_.tile([128, 512], bf, name="a_sb")
            nc.scalar.activation(a_sb[:], sc_ps[:],
                                 mybir.ActivationFunctionType.Exp,
                                 scale=inv_sqrt_hd)
            if ki >= 4 * j:
                nc.vector.tensor_mul(a_sb[:], a_sb[:],
                                     mask_sb[:, ki - 4 * j, :])
            a_tiles[n] = a_sb

        def emit_back(n):
            ti, h, ki, nk, b, j = tiles[n]
            kg = b * KTB + ki
            a_sb = a_tiles.pop(n)
            if ki == 0:
                unit_ps[(ti, h)] = (ps_tile(), ps_tile((1, 512)))
            at_ps, dn_ps = unit_ps[(ti, h)]
            mm(dn_ps[:], ones_sb[:, 0:1], a_sb[:],
               start=(ki == 0), stop=(ki == nk - 1))
            mm(at_ps[:], Vsb[:, kg, :], a_sb[:],
               start=(ki == 0), stop=(ki == nk - 1))
            if ki == nk - 1:
                del unit_ps[(ti, h)]
                dr = smp.tile([1, 512], f32, name="dr")
                nc.vector.reciprocal(dr[:], dn_ps[:])
                rb = smp.tile([128, 512], f32, name="rb")
                nc.gpsimd.partition_broadcast(rb[:], dr[:])
                ATn = anp.tile([128, 512], bf, name="ATn")
                nc.vector.tensor_mul(ATn[:], at_ps[:], rb[:])
                # owner of token half hf is core 2*(ti%4)+hf for this stage.
                # Stores ride the gpsimd SWDGE queue: ordered right after the
                # broadcast, and never stuck behind the Wo/xt streams.
                a2a = a2aA_in if ti < 4 else a2aB_in
                for hf in range(2):
                    nc.gpsimd.dma_start(
                        a2a[2 * (ti % 4) + hf, h * 128:(h + 1) * 128, :],
                        ATn[:, hf * 256:(hf + 1) * 256])

        def fire_a2a(a2a_in_, a2a_out_):
            nc.gpsimd.collective_compute(
                "AllToAll", mybir.AluOpType.bypass,
                replica_groups=[list(range(NCORES))],
                ins=[a2a_in_.opt()], outs=[a2a_out_.opt()],
            )

        def load_atf(a2a_out_, ATf_, eng):
            # assemble [d, fc, t]: global feature chunk fc = 4*src_core + h
            for c in range(NCORES):
                for h in range(QH):
                    eng.dma_start(ATf_[:, 4 * c + h, :],
                                  a2a_out_[c, h * 128:(h + 1) * 128, :])

        def oproj_block(p, Hc, ATf_):
            ops = [ps_tile() for _ in range(2)]
            for fq in range(16):
                wo_sb = wo_get(p * 128 + Hc * 16 + fq)
                for tc in range(2):
                    for k in range(2):
                        fc = fq * 2 + k
                        mm(ops[tc][:],
                           ATf_[:, fc, tc * 128:(tc + 1) * 128],
                           wo_sb[:, k, :],
                           start=(fc == 0), stop=(fc == NFC - 1))
            return ops

        def oproj_drain(p, Hc, ops):
            # bias bo is added on the host after the gather; draining via the
            # Act engine keeps the DVE free for the attention drain chain
            for tc in range(2):
                o_sb = osp.tile([128, 512], f32, name="o_sb")
                nc.scalar.copy(o_sb[:], ops[tc][:])
                nc.scalar.dma_start(
                    out[p * 256 + tc * 128:p * 256 + (tc + 1) * 128,
                        Hc * 512:(Hc + 1) * 512],
                    o_sb[:])

        # pass-A O-projection blocks are interleaved into the tail of phase 2
        # (their features are complete once the stage-A collective lands),
        # filling the PE's exp-wait stalls and relaxing the Wo stream rate.
        # Drains lag one block so the Act queue never head-of-line blocks.
        lastA = sum(1 for t in tiles if t[0] < 4) - 1  # last batch-0 tile
        unit_last = {}
        for n, (ti, h, ki, nk, b, j) in enumerate(tiles):
            if ki == nk - 1:
                unit_last[n] = (ti, h)
        insert_at = {}   # interleaving pass A into phase 2 measured slower
        pending = []

        def after_back(m):
            if m == lastA:
                fire_a2a(a2aA_in, a2aA_out)
                load_atf(a2aA_out, ATfA, nc.sync)
                for i in range(5):   # warm the Wo stream during phase 2
                    wo_prefetch(i)
            u = unit_last.get(m)
            if u is not None and u in insert_at:
                Hc = insert_at[u]
                if pending:
                    oproj_drain(*pending.pop())
                pending.append((0, Hc, oproj_block(0, Hc, ATfA)))

        for n in range(len(tiles)):
            emit_front(n)
            if n >= W:
                emit_back(n - W)
                after_back(n - W)
        for n in range(len(tiles) - W, len(tiles)):
            emit_back(n)
            after_back(n)

        # ==== phase 3: stage-B collective hidden behind pass A =============
        fire_a2a(a2aB_in, a2aB_out)
        for Hc in range(H // 512):
            if Hc == 5:
                # stage-B features land mid-pass-A on the Act queue (the
                # collective is long done, so this never stalls anything)
                load_atf(a2aB_out, ATfB, nc.scalar)
            if pending:
                oproj_drain(*pending.pop())
            pending.append((0, Hc, oproj_block(0, Hc, ATfA)))
        for Hc in range(H // 512):
            if pending:
                oproj_drain(*pending.pop())
            pending.append((1, Hc, oproj_block(1, Hc, ATfB)))
        while pending:
            oproj_drain(*pending.pop())

    nc.compile()
    return nc


def _host_inputs(hidden_states, position_ids, Wq, bq, Wk, bk, Wv, bv, Wo, bo):
    import ml_dtypes
    bf16 = ml_dtypes.bfloat16
    f = np.float32
    X = np.asarray(hidden_states, f).reshape(TOK, H)
    xT = np.ascontiguousarray(X.T).astype(bf16)
    xTt = np.ascontiguousarray(xT.reshape(H, TS, 512).transpose(1, 0, 2))

    pos = np.asarray(position_ids).astype(f).reshape(TOK)
    inv_freq = (1.0 / (THETA ** (np.arange(0, HD, 2, dtype=f) / HD))).astype(f)
    M = inv_freq[:, None] * pos[None, :]              # [64, TOK]
    cosT = np.repeat(np.cos(M), 2, axis=0).astype(f)  # [128, TOK]
    sinT = np.repeat(np.sin(M), 2, axis=0).astype(f)

    rotM = np.zeros((HD, HD), f)
    for i in range(HD // 2):
        rotM[2 * i + 1, 2 * i] = -1.0   # out[2i]   = -in[2i+1]
        rotM[2 * i, 2 * i + 1] = 1.0    # out[2i+1] =  in[2i]

    Wo_f = np.asarray(Wo, f)
    # woQ[Hc, fq, d, k, o] = Wo^T[(fq*2+k)*128+d, Hc*512+o]
    woQ = np.ascontiguousarray(
        Wo_f.T.reshape(16, 2, 128, 8, 512).transpose(3, 0, 2, 1, 4)
    ).astype(bf16)

    shared = {
        "xTt": xTt, "cosT": cosT.astype(bf16), "sinT": sinT.astype(bf16),
        "rotM": rotM.astype(bf16),
        "ident": np.eye(128, dtype=f).astype(bf16),
        "ones1": np.ones((128, 1), bf16),
        "woQ": woQ,
        "maskI": (np.arange(512)[None, None, :]
                  - np.arange(128)[:, None, None]
                  - 128 * np.arange(4)[None, :, None] >= 0).astype(bf16),
    }
    Wq, Wk, Wv = (np.asarray(a, f) for a in (Wq, Wk, Wv))
    bq, bk, bv = (np.asarray(a, f) for a in (bq, bk, bv))
    in_maps = []
    for c in range(NCORES):
        m = dict(shared)
        # [p, h-tile, o] resident layout: wT[h, o] with h = ht*128 + p
        wqT = Wq[c * QO:(c + 1) * QO, :].T.reshape(H // 128, 128, QO)
        m["wqP"] = np.ascontiguousarray(wqT.transpose(1, 0, 2)).astype(bf16)
        wkT = Wk[c * HD:(c + 1) * HD, :].T.reshape(H // 128, 128, HD)
        m["wkP"] = np.ascontiguousarray(wkT.transpose(1, 0, 2)).astype(bf16)
        wvT = Wv[c * HD:(c + 1) * HD, :].T.reshape(H // 128, 128, HD)
        m["wvP"] = np.ascontiguousarray(wvT.transpose(1, 0, 2)).astype(bf16)
        m["bqP"] = np.ascontiguousarray(bq[c * QO:(c + 1) * QO].reshape(QH, 128).T)
        m["bkP"] = bk[c * HD:(c + 1) * HD].reshape(128, 1).copy()
        m["bvP"] = bv[c * HD:(c + 1) * HD].reshape(128, 1).copy()
        in_maps.append(m)
    return in_maps


def kernel(hidden_states, position_ids, Wq, bq, Wk, bk, Wv, bv, Wo, bo):
    global LAST_EXEC_NS, LAST_RESULT
    from concourse.bass_utils import run_bass_kernel_spmd

    if "nc" not in _compiled:
        _compiled["nc"] = _build()
    nc = _compiled["nc"]

    in_maps = _host_inputs(hidden_states, position_ids,
                           Wq, bq, Wk, bk, Wv, bv, Wo, bo)
    trace = os.environ.get("KERNEL_TRACE", "0") == "1"
    res = run_bass_kernel_spmd(nc, in_maps, core_ids=list(range(NCORES)),
                               trace=trace)
    LAST_EXEC_NS = res.exec_time_ns
    LAST_RESULT = res
    # core c: out rows 0:256 = batch-0 tokens c*256.., rows 256:512 =
    # batch-1 tokens 2048+c*256..
    full = np.empty((TOK, H), np.float32)
    for c in range(NCORES):
        o = res.results[c]["out"]
        full[c * 256:(c + 1) * 256] = o[0:256]
        full[2048 + c * 256:2048 + (c + 1) * 256] = o[256:512]
    full += np.asarray(bo, np.float32)[None, :]
    return full.reshape(B, S, H)


# revision 46
# speedup vs baseline: 1.0452x; 1.0452x over previous
"""Trainium2 Bass kernel for Llama-style GQA attention (B=2,S=2048,H=4096,NH=32,NKV=8,HD=128).

Sharding: tensor-parallel over heads for QKV+attention (core c owns Q-heads
4c..4c+3 and GQA KV-head c). For the output projection, cores exchange
attention outputs with a single bf16 AllToAll (each core ends with all 4096
features for its 512 tokens) and then apply the FULL Wo locally, streamed
from HBM. This moves ~4 MB per core over the interconnect instead of the
~67 MB fp32 partial-sum ReduceScatter.

Three GEMM-dense phases keep the PE array continuously busy (p-state ramp):
  1) QKV projection + rope for all 8 token slices
  2) causal attention, software-pipelined across (slice, head) tiles
  3) AllToAll + output projection with Wo streamed from HBM
"""

import math
import os
from contextlib import ExitStack

import numpy as np

B, S, H = 2, 2048, 4096
NH, NKV, HD = 32, 8, 128
THETA = 1000000.0
NCORES = 8
QH = NH // NCORES            # 4 q-heads per core
TOK = B * S                  # 4096 tokens (flattened batch*seq)
QO = QH * HD                 # 512 q-out dims per core
TT = TOK // 128              # 32 token tiles of 128
TS = TOK // 512              # 8 token slices of 512
SB = S // 512                # 4 q-slices of 512 per batch
KTB = S // 128               # 16 k-tiles of 128 per batch
NFC = NH                     # 32 feature chunks of 128 (full Wo contraction)

LAST_EXEC_NS = None
LAST_RESULT = None

_compiled = {}


def _build():
    import concourse.bass as bass
    import concourse.mybir as mybir
    import concourse.tile as tile
    from concourse import bacc

    f32 = mybir.dt.float32
    bf = mybir.dt.bfloat16
    nc = bacc.Bacc("TRN2", target_bir_lowering=False, debug=False,
                   num_devices=NCORES)

    def inp(name, shape, dt=f32):
        return nc.dram_tensor(name, shape, dt, kind="ExternalInput").ap()

    # hidden transposed and host-pre-tiled: xTt[ti] is a contiguous
    # (H, 512) block for token slice ti -> single-burst DMA tiles
    xTt = inp("xTt", (TS, H, 512), bf)
    # QKV weight shards host-permuted to SBUF-resident layout [p, tile, out]
    wqP = inp("wqP", (128, H // 128, QO), bf)
    wkP = inp("wkP", (128, H // 128, HD), bf)
    wvP = inp("wvP", (128, H // 128, HD), bf)
    bqP = inp("bqP", (128, QH))         # bq shard as [d, head]
    bkP = inp("bkP", (128, 1))
    bvP = inp("bvP", (128, 1))
    # FULL Wo^T blocked for streaming: [Hc, fq, d, k, o] with feature
    # f = (fq*2+k)*128 + d and output column o within chunk Hc
    woQ = inp("woQ", (H // 512, 16, 128, 2, 512), bf)
    boB = inp("boB", (128, H), bf)      # full bo broadcast to 128 partitions
    cosT = inp("cosT", (HD, TOK), bf)
    sinT = inp("sinT", (HD, TOK), bf)
    rotM = inp("rotM", (HD, HD), bf)    # lhsT for rotate_half_interleaved
    ident = inp("ident", (128, 128), bf)
    ones1 = inp("ones1", (128, 1), bf)
    maskI = inp("maskI", (128, 4, 512), bf)  # causal diag-tile masks

    # core c's final output: rows 0:256 = batch-0 tokens c*256..(c+1)*256,
    # rows 256:512 = batch-1 tokens 2048+c*256..2048+(c+1)*256
    out = nc.dram_tensor("out", (512, H), f32, kind="ExternalOutput").ap()
    # Two-stage AllToAll of attention outputs (stage A: batch-0 slices 0-3,
    # fired mid-kernel; stage B: batch-1 slices 4-7, fired at the end and
    # hidden behind stage A's output projection). Owner j holds 256 tokens.
    a2aA_in = nc.dram_tensor("a2aA_in", (8, QO, 256), bf, kind="Internal").ap()
    a2aA_out = nc.dram_tensor("a2aA_out", (8, QO, 256), bf, kind="Internal").ap()
    a2aB_in = nc.dram_tensor("a2aB_in", (8, QO, 256), bf, kind="Internal").ap()
    a2aB_out = nc.dram_tensor("a2aB_out", (8, QO, 256), bf, kind="Internal").ap()

    inv_sqrt_hd = 1.0 / math.sqrt(HD)

    def mm(out_, lhsT, rhs, **kw):
        nc.tensor.matmul(out_, lhsT, rhs, **kw)

    with tile.TileContext(nc) as tc, ExitStack() as stk:
        # ---------------- constants + persistent activations ----------------
        cpool = stk.enter_context(tc.tile_pool(name="consts", bufs=1))
        apool = stk.enter_context(tc.tile_pool(name="acts", bufs=1))

        # Preamble loads go through the Activation HW-DGE queue so the xt
        # token stream (sync queue) is never stuck behind them. Weights are
        # interleaved per contraction chunk hi so the slice-0 QKV matmuls
        # start as soon as (wq|wk|wv)[hi=0] + xt tile 0 land (~2us), keeping
        # aggregate HBM demand under the per-core cap during startup.
        wq_res = apool.tile([128, H // 128, QO], bf)
        wk_res = apool.tile([128, H // 128, HD], bf)
        wv_res = apool.tile([128, H // 128, HD], bf)
        for hi in range(H // 128):
            nc.scalar.dma_start(wq_res[:, hi:hi + 1, :], wqP[:, hi:hi + 1, :])
            nc.scalar.dma_start(wk_res[:, hi:hi + 1, :], wkP[:, hi:hi + 1, :])
            nc.scalar.dma_start(wv_res[:, hi:hi + 1, :], wvP[:, hi:hi + 1, :])

        bq_sb = cpool.tile([128, QH], f32)
        nc.scalar.dma_start(bq_sb[:], bqP[:])
        bk_sb = cpool.tile([128, 1], f32)
        nc.scalar.dma_start(bk_sb[:], bkP[:])
        bv_sb = cpool.tile([128, 1], f32)
        nc.scalar.dma_start(bv_sb[:], bvP[:])
        cos_sb = cpool.tile([128, TOK], bf)
        nc.scalar.dma_start(cos_sb[:], cosT[:])
        sin_sb = cpool.tile([128, TOK], bf)
        nc.scalar.dma_start(sin_sb[:], sinT[:])
        rot_sb = cpool.tile([128, 128], bf)
        nc.scalar.dma_start(rot_sb[:], rotM[:])
        id_sb = cpool.tile([128, 128], bf)
        nc.scalar.dma_start(id_sb[:], ident[:])
        ones_sb = cpool.tile([128, 1], bf)
        nc.scalar.dma_start(ones_sb[:], ones1[:])
        bo_bc = cpool.tile([128, H], bf)
        nc.scalar.dma_start(bo_bc[:], boB[:])
        mask_sb = cpool.tile([128, 4, 512], bf)
        nc.scalar.dma_start(mask_sb[:], maskI[:])

        KT = apool.tile([128, TOK], bf)          # K^T (rope'd)
        Vsb = apool.tile([128, TT, 128], bf)     # V in [t mod 128, t tile, d]
        QTs = apool.tile([128, TS, QH, 512], bf)  # all Q^T slices (rope'd)
        ATfA = apool.tile([128, NFC, 256], bf)   # post-A2A features, stage A
        ATfB = apool.tile([128, NFC, 256], bf)   # post-A2A features, stage B

        xp = stk.enter_context(tc.tile_pool(name="xts", bufs=4))
        vp = stk.enter_context(tc.tile_pool(name="vts", bufs=2))
        rp = stk.enter_context(tc.tile_pool(name="ropet", bufs=3))
        ap_ = stk.enter_context(tc.tile_pool(name="asb", bufs=7))
        anp = stk.enter_context(tc.tile_pool(name="atn", bufs=3))
        smp = stk.enter_context(tc.tile_pool(name="smalls", bufs=2))
        wop = stk.enter_context(tc.tile_pool(name="wos", bufs=6))
        osp = stk.enter_context(tc.tile_pool(name="ost", bufs=2))
        pp = stk.enter_context(tc.tile_pool(name="ps", bufs=8, space="PSUM"))

        def ps_tile(shape=(128, 512), dt=f32):
            return pp.tile(list(shape), dt, name="ps", tag="ps")

        # Wo streaming schedule: two full passes (one per A2A stage),
        # linear chunk list with lookahead prefetch + explicit pre-warm
        wo_chunks = [(p, Hc, fq) for p in range(2)
                     for Hc in range(H // 512) for fq in range(16)]
        wo_tiles = {}

        def wo_prefetch(idx):
            if idx < len(wo_chunks) and wo_chunks[idx] not in wo_tiles:
                pf = wop.tile([128, 2, 512], bf, name="wo_sb")
                nc.sync.dma_start(pf[:], woQ[wo_chunks[idx][1],
                                             wo_chunks[idx][2]])
                wo_tiles[wo_chunks[idx]] = pf

        def wo_get(idx):
            key = wo_chunks[idx]
            wo_sb = wo_tiles.pop(key, None)
            if wo_sb is None:
                wo_sb = wop.tile([128, 2, 512], bf, name="wo_sb")
                nc.sync.dma_start(wo_sb[:], woQ[key[1], key[2]])
            for la in (idx + 1, idx + 2, idx + 3, idx + 4):
                wo_prefetch(la)
            return wo_sb

        # =================== phase 1: QKV + rope (all slices) ===============
        def rope_slice(ti):
            t0 = ti * 512
            for q in range(QH + 1):
                ap_slice = KT[:, t0:t0 + 512] if q == QH else QTs[:, ti, q, :]
                rps = ps_tile()
                mm(rps[:], rot_sb[:], ap_slice, start=True, stop=True)
                t1 = rp.tile([128, 512], bf, name="t1")
                nc.vector.tensor_mul(t1[:], ap_slice, cos_sb[:, t0:t0 + 512])
                t2 = rp.tile([128, 512], bf, name="t2")
                nc.vector.tensor_mul(t2[:], rps[:], sin_sb[:, t0:t0 + 512])
                nc.vector.tensor_add(ap_slice, t1[:], t2[:])

        def vt_slice(ti, VTs):
            for s4 in range(4):
                g = ti * 4 + s4
                vps = pp.tile([128, 128], bf, name="vps", tag="ps")
                nc.tensor.transpose(vps[:], VTs[:, s4 * 128:(s4 + 1) * 128],
                                    id_sb[:])
                nc.scalar.copy(Vsb[:, g, :], vps[:])

        prev = None  # (ti, VTs) pending rope+transpose
        for ti in range(TS):
            t0 = ti * 512
            psq = [ps_tile() for _ in range(QH)]
            psk = ps_tile()
            psv = ps_tile()
            for hi in range(H // 128):
                h0 = hi * 128
                xt = xp.tile([128, 512], bf, name="xt")
                nc.sync.dma_start(xt[:], xTt[ti, h0:h0 + 128, :])
                st = (hi == 0)
                en = (hi == H // 128 - 1)
                for q in range(QH):
                    mm(psq[q][:], wq_res[:, hi, q * 128:(q + 1) * 128],
                       xt[:], start=st, stop=en)
                mm(psk[:], wk_res[:, hi, :], xt[:], start=st, stop=en)
                mm(psv[:], wv_res[:, hi, :], xt[:], start=st, stop=en)

            # bias add (per-partition) while draining PSUM
            VTs = vp.tile([128, 512], bf, name="VTs")
            for q in range(QH):
                nc.scalar.add(QTs[:, ti, q, :], psq[q][:], bq_sb[:, q:q + 1])
            nc.scalar.add(KT[:, t0:t0 + 512], psk[:], bk_sb[:, 0:1])
            nc.scalar.add(VTs[:], psv[:], bv_sb[:, 0:1])

            # rope + V-transpose of the PREVIOUS slice fills the PE while
            # this slice's PSUM drains on the scalar engine
            if prev is not None:
                rope_slice(prev[0])
                vt_slice(prev[0], prev[1])
            prev = (ti, VTs)
        rope_slice(prev[0])
        vt_slice(prev[0], prev[1])

        # ============ phase 2: causal attention, software-pipelined =========
        # global tile list over (slice, head, k-tile); at/dn matmuls lag the
        # score/exp stream by W tiles so the PE never waits on the Act engine
        W = 6
        tiles = []
        for ti in range(TS):
            b, j = ti // SB, ti % SB
            nk = 4 * j + 4
            for h in range(QH):
                for ki in range(nk):
                    tiles.append((ti, h, ki, nk, b, j))

        unit_ps = {}   # (ti,h) -> (at_ps, dn_ps)
        a_tiles = {}   # index -> exp'd score tile in SBUF

        def emit_front(n):
            ti, h, ki, nk, b, j = tiles[n]
            kg = b * KTB + ki
            sc_ps = ps_tile()
            mm(sc_ps[:], KT[:, kg * 128:(kg + 1) * 128],
               QTs[:, ti, h, :], start=True, stop=True)
            a_sb = ap_.tile([128, 512], bf, name="a_sb")
            nc.scalar.activation(a_sb[:], sc_ps[:],
                                 mybir.ActivationFunctionType.Exp,
                                 scale=inv_sqrt_hd)
            if ki >= 4 * j:
                nc.vector.tensor_mul(a_sb[:], a_sb[:],
                                     mask_sb[:, ki - 4 * j, :])
            a_tiles[n] = a_sb

        def emit_back(n):
            ti, h, ki, nk, b, j = tiles[n]
            kg = b * KTB + ki
            a_sb = a_tiles.pop(n)
            if ki == 0:
                unit_ps[(ti, h)] = (ps_tile(), ps_tile((1, 512)))
            at_ps, dn_ps = unit_ps[(ti, h)]
            mm(dn_ps[:], ones_sb[:, 0:1], a_sb[:],
               start=(ki == 0), stop=(ki == nk - 1))
            mm(at_ps[:], Vsb[:, kg, :], a_sb[:],
               start=(ki == 0), stop=(ki == nk - 1))
            if ki == nk - 1:
                del unit_ps[(ti, h)]
                dr = smp.tile([1, 512], f32, name="dr")
                nc.vector.reciprocal(dr[:], dn_ps[:])
                rb = smp.tile([128, 512], f32, name="rb")
                nc.gpsimd.partition_broadcast(rb[:], dr[:])
                ATn = anp.tile([128, 512], bf, name="ATn")
                nc.vector.tensor_mul(ATn[:], at_ps[:], rb[:])
                # owner of token half hf is core 2*(ti%4)+hf for this stage.
                # Stores ride the gpsimd SWDGE queue: ordered right after the
                # broadcast, and never stuck behind the Wo/xt streams.
                a2a = a2aA_in if ti < 4 else a2aB_in
                for hf in range(2):
                    nc.gpsimd.dma_start(
                        a2a[2 * (ti % 4) + hf, h * 128:(h + 1) * 128, :],
                        ATn[:, hf * 256:(hf + 1) * 256])

        def fire_a2a(a2a_in_, a2a_out_):
            nc.gpsimd.collective_compute(
                "AllToAll", mybir.AluOpType.bypass,
                replica_groups=[list(range(NCORES))],
                ins=[a2a_in_.opt()], outs=[a2a_out_.opt()],
            )

        def load_atf(a2a_out_, ATf_, eng):
            # assemble [d, fc, t]: global feature chunk fc = 4*src_core + h
            for c in range(NCORES):
                for h in range(QH):
                    eng.dma_start(ATf_[:, 4 * c + h, :],
                                  a2a_out_[c, h * 128:(h + 1) * 128, :])

        def oproj_block(p, Hc, ATf_):
            ops = [ps_tile() for _ in range(2)]
            for fq in range(16):
                wo_sb = wo_get(p * 128 + Hc * 16 + fq)
                for tc in range(2):
                    for k in range(2):
                        fc = fq * 2 + k
                        mm(ops[tc][:],
                           ATf_[:, fc, tc * 128:(tc + 1) * 128],
                           wo_sb[:, k, :],
                           start=(fc == 0), stop=(fc == NFC - 1))
            return ops

        def oproj_drain(p, Hc, ops):
            for tc in range(2):
                o_sb = osp.tile([128, 512], f32, name="o_sb")
                nc.vector.tensor_add(o_sb[:], ops[tc][:],
                                     bo_bc[:, Hc * 512:(Hc + 1) * 512])
                nc.scalar.dma_start(
                    out[p * 256 + tc * 128:p * 256 + (tc + 1) * 128,
                        Hc * 512:(Hc + 1) * 512],
                    o_sb[:])

        # pass-A O-projection blocks are interleaved into the tail of phase 2
        # (their features are complete once the stage-A collective lands),
        # filling the PE's exp-wait stalls and relaxing the Wo stream rate.
        # Drains lag one block so the Act queue never head-of-line blocks.
        lastA = sum(1 for t in tiles if t[0] < 4) - 1  # last batch-0 tile
        unit_last = {}
        for n, (ti, h, ki, nk, b, j) in enumerate(tiles):
            if ki == nk - 1:
                unit_last[n] = (ti, h)
        insert_at = {}   # interleaving pass A into phase 2 measured slower
        pending = []

        def after_back(m):
            if m == lastA:
                fire_a2a(a2aA_in, a2aA_out)
                load_atf(a2aA_out, ATfA, nc.sync)
                for i in range(5):   # warm the Wo stream during phase 2
                    wo_prefetch(i)
            u = unit_last.get(m)
            if u is not None and u in insert_at:
                Hc = insert_at[u]
                if pending:
                    oproj_drain(*pending.pop())
                pending.append((0, Hc, oproj_block(0, Hc, ATfA)))

        for n in range(len(tiles)):
            emit_front(n)
            if n >= W:
                emit_back(n - W)
                after_back(n - W)
        for n in range(len(tiles) - W, len(tiles)):
            emit_back(n)
            after_back(n)

        # ==== phase 3: stage-B collective hidden behind pass A =============
        fire_a2a(a2aB_in, a2aB_out)
        for p, ATf_ in ((0, ATfA), (1, ATfB)):
            for Hc in range(H // 512):
                if p == 0 and Hc == 5:
                    # stage-B features land mid-pass-A on the Act queue (the
                    # collective is long done, so this never stalls anything)
                    load_atf(a2aB_out, ATfB, nc.scalar)
                ops = oproj_block(p, Hc, ATf_)
                oproj_drain(p, Hc, ops)

    nc.compile()
    return nc


def _host_inputs(hidden_states, position_ids, Wq, bq, Wk, bk, Wv, bv, Wo, bo):
    import ml_dtypes
    bf16 = ml_dtypes.bfloat16
    f = np.float32
    X = np.asarray(hidden_states, f).reshape(TOK, H)
    xT = np.ascontiguousarray(X.T).astype(bf16)
    xTt = np.ascontiguousarray(xT.reshape(H, TS, 512).transpose(1, 0, 2))

    pos = np.asarray(position_ids).astype(f).reshape(TOK)
    inv_freq = (1.0 / (THETA ** (np.arange(0, HD, 2, dtype=f) / HD))).astype(f)
    M = inv_freq[:, None] * pos[None, :]              # [64, TOK]
    cosT = np.repeat(np.cos(M), 2, axis=0).astype(f)  # [128, TOK]
    sinT = np.repeat(np.sin(M), 2, axis=0).astype(f)

    rotM = np.zeros((HD, HD), f)
    for i in range(HD // 2):
        rotM[2 * i + 1, 2 * i] = -1.0   # out[2i]   = -in[2i+1]
        rotM[2 * i, 2 * i + 1] = 1.0    # out[2i+1] =  in[2i]

    Wo_f = np.asarray(Wo, f)
    # woQ[Hc, fq, d, k, o] = Wo^T[(fq*2+k)*128+d, Hc*512+o]
    woQ = np.ascontiguousarray(
        Wo_f.T.reshape(16, 2, 128, 8, 512).transpose(3, 0, 2, 1, 4)
    ).astype(bf16)

    shared = {
        "xTt": xTt, "cosT": cosT.astype(bf16), "sinT": sinT.astype(bf16),
        "rotM": rotM.astype(bf16),
        "ident": np.eye(128, dtype=f).astype(bf16),
        "ones1": np.ones((128, 1), bf16),
        "woQ": woQ,
        "boB": np.broadcast_to(np.asarray(bo, f).reshape(1, H),
                               (128, H)).astype(bf16).copy(),
        "maskI": (np.arange(512)[None, None, :]
                  - np.arange(128)[:, None, None]
                  - 128 * np.arange(4)[None, :, None] >= 0).astype(bf16),
    }
    Wq, Wk, Wv = (np.asarray(a, f) for a in (Wq, Wk, Wv))
    bq, bk, bv = (np.asarray(a, f) for a in (bq, bk, bv))
    in_maps = []
    for c in range(NCORES):
        m = dict(shared)
        # [p, h-tile, o] resident layout: wT[h, o] with h = ht*128 + p
        wqT = Wq[c * QO:(c + 1) * QO, :].T.reshape(H // 128, 128, QO)
        m["wqP"] = np.ascontiguousarray(wqT.transpose(1, 0, 2)).astype(bf16)
        wkT = Wk[c * HD:(c + 1) * HD, :].T.reshape(H // 128, 128, HD)
        m["wkP"] = np.ascontiguousarray(wkT.transpose(1, 0, 2)).astype(bf16)
        wvT = Wv[c * HD:(c + 1) * HD, :].T.reshape(H // 128, 128, HD)
        m["wvP"] = np.ascontiguousarray(wvT.transpose(1, 0, 2)).astype(bf16)
        m["bqP"] = np.ascontiguousarray(bq[c * QO:(c + 1) * QO].reshape(QH, 128).T)
        m["bkP"] = bk[c * HD:(c + 1) * HD].reshape(128, 1).copy()
        m["bvP"] = bv[c * HD:(c + 1) * HD].reshape(128, 1).copy()
        in_maps.append(m)
    return in_maps


def kernel(hidden_states, position_ids, Wq, bq, Wk, bk, Wv, bv, Wo, bo):
    global LAST_EXEC_NS, LAST_RESULT
    from concourse.bass_utils import run_bass_kernel_spmd

    if "nc" not in _compiled:
        _compiled["nc"] = _build()
    nc = _compiled["nc"]

    in_maps = _host_inputs(hidden_states, position_ids,
                           Wq, bq, Wk, bk, Wv, bv, Wo, bo)
    trace = os.environ.get("KERNEL_TRACE", "0") == "1"
    res = run_bass_kernel_spmd(nc, in_maps, core_ids=list(range(NCORES)),
                               trace=trace)
    LAST_EXEC_NS = res.exec_time_ns
    LAST_RESULT = res
    # core c: out rows 0:256 = batch-0 tokens c*256.., rows 256:512 =
    # batch-1 tokens 2048+c*256..
    full = np.empty((TOK, H), np.float32)
    for c in range(NCORES):
        o = res.results[c]["out"]
        full[c * 256:(c + 1) * 256] = o[0:256]
        full[2048 + c * 256:2048 + (c + 1) * 256] = o[256:512]
    return full.reshape(B, S, H)
